# revision 53
# baseline (speedup 1.0000x reference)
"""Causal self-attention (B=2, S=2048, D=2048, H=16) on 8 TRN2 NeuronCores.

Sharding (data + tensor parallel, per the head-group hint):
  core c -> batch b = c // 4, head group g = c % 4 (heads 4g..4g+3).
  wq/wk/wv are split column-wise per head group (512 cols), wo row-wise
  (512 rows). Each core computes attention for its 4 heads on its batch and
  produces a partial output projection; the host sums the 4 partials per
  batch (the tensor-parallel all-reduce, done at gather time).

Layouts (feature-major so matmuls consume operands natively):
  QT[c,s] = wq.T @ x.T           (lhsT=wq,  rhs=xT)
  KT[c,s] = wk.T @ x.T
  V1[s,(c,1)] = [x @ wv | 1]     (lhsT=xT,  rhs=wv; ones col appended)
  ST[k,q] = K_h Q_h^T            (lhsT=KT_h, rhs=QT_h)
  PT[k,q] = exp(ST*scale - 4 + causal_mask)              (ACT engine)
  AVR[q,(hd,1)] = PT_chunk^T @ V1_h   (lhsT=PT cols, rhs=V1_h): the ones
      column of V1 makes the last output column the softmax denominator,
      so no separate ones-matmul is needed (saves 512 PE cycles/step).
  OTQ[q,hd] = AVR[:, :hd] * (1/AVR[:, hd])   (DVE per-partition scalar)
  OT[hd,q] = PE-transpose(OTQ)   (identity-matmul transpose, phase 3)
  out      = OT^T @ wo           (lhsT=OT, rhs=wo)
Compute dtype fp16; softmax statistics and PSUM accumulation in fp32.

Scheduling:
  - Input DMAs on the sync + gpsimd queues in first-need order (the DMA
    engines are one ~350 GB/s serial pipe); the scalar (ACT) queue is
    kept free because exp dispatch bounds phase 2. q-projection groups
    run first so startup needs only wq + xt.
  - Phase 2 pipelined: sc/exp pairs run 3 steps ahead of the AVR
    matmuls; one paired exp per two kb steps halves ACT overhead.
  - Diagonal trimming: scores/exp skip fully-masked columns; Pool
    applies the causal mask as post-exp zeroing on probs.
  - All SBUF pools are allocated once, outside the repeat loop, so a
    repeat's input prefetch overlaps the previous repeat's phases 2-3.
  - Output partials written fp16 (host accumulates in fp32).
"""

import math

import numpy as np

B = 2
S = 2048
D = 2048
H = 16
HD = 128
N_CORES = 8
NH = 4          # heads per core
C = NH * HD     # 512 per-core projection width
P = 128
DO = D // P     # 16 contraction subtiles
SBLK = 512      # matmul moving free dim / PSUM bank
NSB = S // SBLK  # 4 sequence blocks
NKB = S // P     # 16 key blocks
SCALE = 1.0 / math.sqrt(HD)
EBIAS = -4.0    # constant shift inside exp; cancels in softmax ratio

_STATE = {}


def _build_kernel(repeat=1):
    import concourse.bacc as bacc
    import concourse.mybir as mybir
    import concourse.tile as tile
    from concourse.bass import ts

    F16 = mybir.dt.float16
    F32 = mybir.dt.float32

    nc = bacc.Bacc("TRN2", target_bir_lowering=False, debug=False)

    xt_d = nc.dram_tensor("xt", [D, S], F16, kind="ExternalInput").ap()
    wq_d = nc.dram_tensor("wq", [D, C], F16, kind="ExternalInput").ap()
    wk_d = nc.dram_tensor("wk", [D, C], F16, kind="ExternalInput").ap()
    wv_d = nc.dram_tensor("wv", [D, C], F16, kind="ExternalInput").ap()
    wo_d = nc.dram_tensor("wo", [C, D], F16, kind="ExternalInput").ap()
    id_d = nc.dram_tensor("ident", [P, P], F16, kind="ExternalInput").ap()
    out_d = nc.dram_tensor("out", [S, D], F16, kind="ExternalOutput").ap()

    with tile.TileContext(nc) as tc:
        with tc.tile_pool(name="persist", bufs=1) as p_per:
            # kt split per head, v1 per kb-quadrant, otq per q-block: tile
            # dependencies are tracked per-tile, so consumers at phase
            # boundaries must not falsely wait on the last writer of an
            # unrelated slice
            qt = p_per.tile([P, NH, S], F16)
            kts = [p_per.tile([P, S], F16, name=f"kt{h}") for h in range(NH)]
            v1s = [p_per.tile([P, 4, NH, HD + 1], F16, name=f"v1{m}")
                   for m in range(4)]
            otqs = [p_per.tile([P, 4, NH, HD], F16, name=f"otq{qb}")
                    for qb in range(NSB)]
            ident = p_per.tile([P, P], F16)
            ebias = p_per.tile([P, 1], F32)

            nc.gpsimd.memset(ebias[:], EBIAS)
            for m in range(4):
                nc.gpsimd.memset(v1s[m][:, :, :, HD:HD + 1], 1.0)

            # All SBUF tiles live for the whole program (allocated once,
            # outside the repeat loop): a repeat's input DMA into a tile
            # then only waits for the PREVIOUS repeat's readers of that
            # same tile (phase-1 matmuls, which die early), so the next
            # repeat's input stream overlaps the current repeat's phases
            # 2-3 instead of serializing behind them.
            with tc.tile_pool(name="xw", bufs=1) as p_xw, \
                 tc.tile_pool(name="p2w", bufs=4) as p2w, \
                 tc.tile_pool(name="p2stat", bufs=2) as p2stat, \
                 tc.tile_pool(name="p3t", bufs=8) as p3t, \
                 tc.tile_pool(name="p3stage", bufs=4) as p3stage:
              xts = [p_xw.tile([P, S], F16, tag=f"xt{do}", name=f"xt{do}")
                     for do in range(DO)]
              wq_sb = p_xw.tile([P, DO, C], F16, tag="wq")
              wk_sb = p_xw.tile([P, DO, C], F16, tag="wk")
              wv_sb = p_xw.tile([P, DO, C], F16, tag="wv")
              wo_sb = p_xw.tile([P, NH, D], F16, tag="wo")
              xt_r = xt_d.rearrange("(do p) s -> do p s", p=P)
              wq_r = wq_d.rearrange("(do p) c -> p do c", p=P)
              wk_r = wk_d.rearrange("(do p) c -> p do c", p=P)
              wv_r = wv_d.rearrange("(do p) c -> p do c", p=P)
              for _rep in range(repeat):
                # ---------------- Phase 1: QKV projections ----------------
                with tc.tile_pool(name="p1ps", bufs=6, space="PSUM") as p1ps, \
                     tc.tile_pool(name="p1psl", bufs=2, space="PSUM") as p1psl:
                    # The DMA engines drain the two HWDGE queues round-robin
                    # into one ~350 GB/s serial pipe, so global issue order
                    # == arrival order, and each dma_start costs ~1.3 us of
                    # issuing-sequencer time (so keep transfers >= 512 KB).
                    # Groups are ordered q, k, v (q-only chunks first), so
                    # the startup critical path is just wq (2 MB) + xt
                    # (8 MB) interleaved by first need; wk/wv/wo follow.
                    _di = 0

                    def _dma(dst, src):
                        # Repeat 0 issues on the two fast HWDGE queues (ACT
                        # is idle at program start). Later repeats' input
                        # DMAs are hoisted by the scheduler into the
                        # PREVIOUS repeat's phases 2-3 (prefetch), and an
                        # issue occupies the sequencer 1.3-4 us — so they
                        # use sync + gpsimd only, keeping the scalar (ACT)
                        # sequencer free for the exp dispatch that bounds
                        # phase 2.
                        nonlocal _di
                        if _rep == 0:
                            eng = nc.scalar if _di % 2 == 0 else nc.sync
                        else:
                            eng = nc.sync if _di % 2 == 0 else nc.gpsimd
                        _di += 1
                        eng.dma_start(dst, src)

                    def wchunk(w_sb, w_r, dc):
                        _dma(w_sb[:, dc:dc + 4, :], w_r[:, dc:dc + 4, :])

                    # startup-critical pair heads the sync queue (HWDGE
                    # issue is fast); xt0's st2/st3 half is only needed by
                    # chunk 2 (~55 us in), so it goes late on gpsimd
                    _dma(wq_sb[:, 0:2, :], wq_r[:, 0:2, :])
                    _dma(xts[0][:, 0:2 * SBLK], xt_r[0][:, 0:2 * SBLK])
                    (nc.gpsimd if _rep else nc.scalar).dma_start(
                        xts[0][:, 2 * SBLK:], xt_r[0][:, 2 * SBLK:])
                    _dma(wq_sb[:, 2:4, :], wq_r[:, 2:4, :])
                    _dma(xts[1][:], xt_r[1])
                    wchunk(wq_sb, wq_r, 4)
                    for do in range(2, 5):
                        _dma(xts[do][:], xt_r[do])
                    wchunk(wq_sb, wq_r, 8)
                    for do in range(5, 8):
                        _dma(xts[do][:], xt_r[do])
                    wchunk(wq_sb, wq_r, 12)
                    for do in range(8, DO):
                        _dma(xts[do][:], xt_r[do])
                    for dc in range(0, DO, 4):
                        wchunk(wk_sb, wk_r, dc)
                    for dc in range(0, DO, 4):
                        wchunk(wv_sb, wv_r, dc)
                    nc.sync.dma_start(ident[:], id_d)
                    nc.sync.dma_start(
                        wo_sb[:], wo_d.rearrange("(cs p) d -> p cs d", p=P))

                    # All q groups first (chunk 0 needs only wq + xt, and
                    # 8-wide chunks consume xt tiles at ~DMA delivery pace),
                    # then k, then v; 8-wide chunks use all 8 PSUM banks.
                    groups = []
                    for st in range(NSB):
                        for ct in range(NH):
                            groups.append(("q", ct, st))
                    for st in range(NSB):
                        for ct in range(NH):
                            groups.append(("k", ct, st))
                    for sv in range(NKB):
                        groups.append(("v", sv, 0))

                    # taper the tail so the final copy burst (which gates
                    # phase 2's first PSUM-bank reuse) is short
                    # repeat 0 fills all 8 banks in chunk 0 so startup
                    # consumption paces the cold DMA stream; later repeats
                    # (inputs prefetched) favor the early-freeing 6/2 split
                    if _rep == 0:
                        chunk_sizes = [8, 6, 6, 6, 6, 6, 6, 2, 2]
                    else:
                        chunk_sizes = [6, 6, 6, 6, 6, 6, 6, 2, 2, 2]
                    assert sum(chunk_sizes) == len(groups)
                    chunk_bounds = []
                    gstart = 0
                    for csz in chunk_sizes:
                        chunk_bounds.append((gstart, gstart + csz))
                        gstart += csz
                    for gstart, gend in chunk_bounds:
                        chunk = groups[gstart:gend]
                        # chunk 0 spans both pools (8 banks) so startup
                        # consumption paces DMA delivery; the taper chunks
                        # draw only from the 2-bank pool so the 6 banks
                        # phase 2 reopens first are freed two chunks early
                        psums = []
                        for gi, (kind, i0, i1) in enumerate(chunk):
                            if gstart == 0 and _rep == 0:
                                pool = p1ps if gi < 6 else p1psl
                            elif gstart >= (44 if _rep == 0 else 42):
                                pool = p1psl
                                assert len(chunk) <= 2
                            else:
                                pool = p1ps
                                assert len(chunk) <= 6
                            psums.append(pool.tile([P, SBLK], F32, tag="p1", name="p1ps"))
                        for do in range(DO):
                            for gi, (kind, i0, i1) in enumerate(chunk):
                                first = do == 0
                                last = do == DO - 1
                                if kind == "q":
                                    nc.tensor.matmul(
                                        psums[gi][:],
                                        wq_sb[:, do, ts(i0, P)],
                                        xts[do][:, ts(i1, SBLK)],
                                        start=first, stop=last)
                                elif kind == "k":
                                    nc.tensor.matmul(
                                        psums[gi][:],
                                        wk_sb[:, do, ts(i0, P)],
                                        xts[do][:, ts(i1, SBLK)],
                                        start=first, stop=last)
                                else:
                                    nc.tensor.matmul(
                                        psums[gi][:],
                                        xts[do][:, ts(i0, P)],
                                        wv_sb[:, do, :],
                                        start=first, stop=last)
                        for gi, (kind, i0, i1) in enumerate(chunk):
                            if kind == "q":
                                nc.any.tensor_copy(qt[:, i0, ts(i1, SBLK)], psums[gi][:])
                            elif kind == "k":
                                nc.any.tensor_copy(kts[i0][:, ts(i1, SBLK)], psums[gi][:])
                            else:
                                nc.any.tensor_copy(
                                    v1s[i0 // 4][:, i0 % 4, :, 0:HD],
                                    psums[gi][:].rearrange("p (h d) -> p h d", h=NH))

                # ---------------- Phase 2: causal attention ----------------
                # Pipelined depth 3: sc/exp for steps i+1..i+3 are emitted
                # before the AVR matmuls for step i, so the PE always has
                # independent work while ACT computes the exp it needs next.
                with tc.tile_pool(name="ps_s", bufs=2, space="PSUM") as ps_s, \
                     tc.tile_pool(name="ps_av", bufs=1, space="PSUM") as ps_av:

                    steps = []
                    for qb in range(NSB):
                        nkb = 4 * (qb + 1)  # causal: only key blocks <= q blk
                        for h in range(NH):
                            for kb in range(nkb):
                                steps.append((qb, h, kb, nkb))

                    avrs = {}

                    def emit_sc_pair(s0, s1):
                        # one [P,2,SBLK] two-bank PSUM tile per kb pair and a
                        # single paired exp: halves the per-instruction ACT
                        # overhead that bounds phase 2. The causal mask is
                        # applied AFTER the exp, as a triangular zeroing of
                        # the probs tile by the (otherwise idle) Pool engine:
                        # exp of raw scores is bounded (|scaled| <~ 5), and
                        # exact zeros in probs contribute exactly nothing to
                        # the AV/rsum accumulation. This keeps DVE and PSUM
                        # out of the sc->exp->avr chain entirely.
                        qb, h, kb0, nkb = s0
                        scp = ps_s.tile([P, 2, SBLK], F32, tag="sc", name="scp")
                        probs = p2w.tile([P, 2, SBLK], F16, tag="probs", name="probsp")
                        w0s = []
                        for kb_, nkb_ in ((s0[2], s0[3]), (s1[2], s1[3])):
                            a = kb_ - 4 * qb if kb_ >= nkb_ - 4 else None
                            w0s.append((a, a * P if a else 0))
                        # both halves compute the shared column range
                        # [wp:512] so the paired exp never reads PSUM the
                        # matmuls didn't write; the over-computed columns of
                        # the larger-a half are zeroed with the mask below
                        wp = min(w0 for a, w0 in w0s)
                        for i, kb_ in enumerate((s0[2], s1[2])):
                            nc.tensor.matmul(
                                scp[:, i, wp:],
                                kts[h][:, ts(kb_, P)],
                                qt[:, h, qb * SBLK + wp:(qb + 1) * SBLK],
                                start=True, stop=True)
                        nc.scalar.activation(
                            probs[:, :, wp:], scp[:, :, wp:],
                            mybir.ActivationFunctionType.Exp,
                            bias=ebias[:], scale=SCALE)
                        for i, (a, w0) in enumerate(w0s):
                            if a is not None:
                                # causal mask on probs: zero the fully-masked
                                # columns, then zero the upper triangle of
                                # the 128-wide diagonal block (keep j-p >= 0
                                # within it)
                                if w0:
                                    nc.gpsimd.memset(probs[:, i, 0:w0], 0.0)
                                nc.gpsimd.affine_select(
                                    out=probs[:, i, w0:w0 + P],
                                    in_=probs[:, i, w0:w0 + P],
                                    compare_op=mybir.AluOpType.is_ge,
                                    fill=0.0,
                                    base=0,
                                    channel_multiplier=-1,
                                    pattern=[[1, P]],
                                )
                        return [(s0, probs[:, 0, :]), (s1, probs[:, 1, :])]

                    def emit_avrs(step, probs):
                        qb, h, kb, nkb = step
                        if kb == 0:
                            # one PSUM bank per chunk: accumulation groups
                            # must not share a bank (start zeroes the bank)
                            avrs[(qb, h)] = [
                                ps_av.tile([P, HD + 1], F32, tag=f"avr{c}",
                                           name=f"avr{c}")
                                for c in range(4)]
                        tiles = avrs[(qb, h)]
                        for c in range(4):
                            nc.tensor.matmul(
                                tiles[c][:],
                                probs[:, ts(c, P)],
                                v1s[kb // 4][:, kb % 4, h, :],
                                start=(kb == 0), stop=(kb == nkb - 1))
                        if kb == nkb - 1:
                            # reciprocal + per-partition-scalar multiply (the
                            # DVE ALU has no divide: TensorScalarPtr/divide
                            # fails the compiler ISA check)
                            rcp = p2stat.tile([P, 4], F32, tag="rcp")
                            for c in range(4):
                                nc.vector.reciprocal(
                                    rcp[:, c:c + 1], tiles[c][:, HD:HD + 1])
                            for c in range(4):
                                nc.vector.tensor_scalar_mul(
                                    out=otqs[qb][:, c, h, :],
                                    in0=tiles[c][:, 0:HD],
                                    scalar1=rcp[:, c:c + 1])
                            del avrs[(qb, h)]

                    from collections import deque
                    pend = deque()
                    for j in range(0, len(steps), 2):
                        pend.extend(emit_sc_pair(steps[j], steps[j + 1]))
                        while len(pend) > 6:
                            emit_avrs(*pend.popleft())
                    while pend:
                        emit_avrs(*pend.popleft())

                # ---------------- Phase 3: output projection ----------------
                # PE-transpose OTQ[q,hd] tiles back to OT[hd,q] for the
                # output matmul; transposes for so-block n+2 are emitted
                # ahead of the po matmuls for block n (ACT does the
                # PSUM->SBUF copies, DVE the stage copies)
                with tc.tile_pool(name="p3tp", bufs=4, space="PSUM") as p3tp, \
                     tc.tile_pool(name="p3ps", bufs=4, space="PSUM") as p3ps:
                    trs = {}

                    def emit_tr(so):
                        for cs in range(NH):
                            tp = p3tp.tile([P, P], F16, tag="tp")
                            nc.tensor.transpose(
                                tp[:], otqs[so // 4][:, so % 4, cs, :],
                                ident[:])
                            tr = p3t.tile([P, P], F16, tag="tr")
                            nc.scalar.activation(
                                tr[:], tp[:],
                                mybir.ActivationFunctionType.Copy)
                            trs[(so, cs)] = tr

                    emit_tr(0)
                    emit_tr(1)
                    for so in range(NKB):
                        for no in range(NSB):
                            po = p3ps.tile([P, SBLK], F32, tag="po")
                            for cs in range(NH):
                                nc.tensor.matmul(
                                    po[:],
                                    trs[(so, cs)][:],
                                    wo_sb[:, cs, ts(no, SBLK)],
                                    start=(cs == 0), stop=(cs == NH - 1))
                            if no == 0 and so + 2 < NKB:
                                emit_tr(so + 2)
                            stage = p3stage.tile([P, SBLK], F16, tag="st")
                            nc.vector.tensor_copy(stage[:], po[:])
                            deng = nc.sync if (so * NSB + no) % 2 == 0 else nc.scalar
                            deng.dma_start(
                                out_d[ts(so, P), ts(no, SBLK)], stage[:])
                        for cs in range(NH):
                            del trs[(so, cs)]

    nc.compile()
    return nc


def _shard_inputs(x, wq, wk, wv, wo):
    ident = np.eye(P, dtype=np.float16)
    in_maps = []
    for c in range(N_CORES):
        b, g = divmod(c, NH)
        cols = slice(g * C, (g + 1) * C)
        in_maps.append({
            "xt": np.ascontiguousarray(x[b].T).astype(np.float16),
            "wq": wq[:, cols].astype(np.float16),
            "wk": wk[:, cols].astype(np.float16),
            "wv": wv[:, cols].astype(np.float16),
            "wo": np.ascontiguousarray(wo[cols, :]).astype(np.float16),
            "ident": ident,
        })
    return in_maps


def kernel(x, wq, wk, wv, wo):
    from concourse.bass_utils import run_bass_kernel_spmd

    if "nc" not in _STATE:
        _STATE["nc"] = _build_kernel()
    nc = _STATE["nc"]

    in_maps = _shard_inputs(
        np.asarray(x), np.asarray(wq), np.asarray(wk),
        np.asarray(wv), np.asarray(wo))
    res = run_bass_kernel_spmd(nc, in_maps, core_ids=list(range(N_CORES)))
    out = np.zeros((B, S, D), dtype=np.float32)
    for c in range(N_CORES):
        b = c // NH
        out[b] += res.results[c]["out"].astype(np.float32)
    return out
